# revision 6
# baseline (speedup 1.0000x reference)
"""ComplexityAwareAttention Trainium2 Bass kernel.

Sharding: 8 cores = 2 batches x 4 head-groups (3 heads each). Each core
computes q/k/v projections for its 3 heads, masked-key-gathered attention
(keys with attention_mask==0 are removed on host - softmax over the kept
keys is mathematically identical), and a partial output projection
(2048, 768). Host sums the 4 partials per batch and adds the fused output
bias (bo + Wo @ bv).

Numerics: fp16 for x / projection weights / q / k / onT / Wo / output
partials (validated 1.5e-3 rel err vs the f32 reference); f32r for the
exp'd scores and v (exp output can exceed fp16 range); f32 PSUM.

Scheduling: the attention inner loop is gated by ScalarE's exp
throughput; PE micro-stalls there drop it to the mid pstate (1.2 GHz)
and double the matmul cost. To keep the PE continuously busy it gets
filler matmul units interleaved one-per-iteration: q-projection of the
second query half inside the first attention head, and the first half's
output projection inside the second half's attention heads.
Normalization runs in 512-col chunks (copy PSUM denom row -> DVE
reciprocal_approx_fast -> Pool partition_broadcast -> DVE multiply) so
the final output projection starts per-chunk instead of per-half.
All HBM tensors are pre-packed on host into the exact SBUF layout
(partition-major) so every DMA moves large contiguous per-partition runs.
Only the Exp activation table is ever loaded (preloaded by a dummy exp
at t=0). No max-subtraction in softmax (|scores| << 80, exp cannot
overflow in f32).
"""

import math
import os
from contextlib import ExitStack

import numpy as np

import concourse.bass as bass
from concourse import bacc
import concourse.mybir as mybir
import concourse.tile as tile
from concourse.bass import ds, ts
from concourse.bass_utils import run_bass_kernel_spmd

F32 = mybir.dt.float32
F32R = mybir.dt.float32r
F16 = mybir.dt.float16
AFT = mybir.ActivationFunctionType

B = 2
S = 2048
D = 768
H = 12
HD = 64
NH = 3  # heads per core
KT_D = D // 128  # 6 contraction tiles over d_model

LAST_EXEC_TIME_NS = None
LAST_RESULTS = None


def build_nc(nk_t):
    n_k = nk_t * 128
    nkc = (n_k + 511) // 512  # xkT 512-col chunks
    nkp = nkc * 512  # padded key columns
    nc = bacc.Bacc(None, target_bir_lowering=False)

    d_xT = nc.dram_tensor("xT", (128, 2, KT_D, 1024), F16, kind="ExternalInput")
    d_xkT = nc.dram_tensor("xkT", (128, nkc, KT_D, 512), F16, kind="ExternalInput")
    d_wq = nc.dram_tensor("wq", (128, KT_D, 192), F16, kind="ExternalInput")
    d_wk = nc.dram_tensor("wk", (128, KT_D, 192), F16, kind="ExternalInput")
    d_wv = nc.dram_tensor("wv", (128, KT_D, 192), F16, kind="ExternalInput")
    d_wo = nc.dram_tensor("wo", (128, 2, D), F16, kind="ExternalInput")
    d_bq = nc.dram_tensor("bq", (128, 2), F32, kind="ExternalInput")
    d_bk = nc.dram_tensor("bk", (128, 2), F32, kind="ExternalInput")
    d_vcol = nc.dram_tensor("vcol", (128, nk_t), F32R, kind="ExternalInput")
    d_out = nc.dram_tensor("out", (128, 16, D), F16, kind="ExternalOutput")

    with ExitStack() as ctx:
        tc = ctx.enter_context(tile.TileContext(nc))
        singles = ctx.enter_context(tc.tile_pool(name="singles", bufs=1))
        expp = ctx.enter_context(tc.tile_pool(name="expp", bufs=4))
        outp = ctx.enter_context(tc.tile_pool(name="outp", bufs=2))
        rowp = ctx.enter_context(tc.tile_pool(name="rowp", bufs=2))
        psp = ctx.enter_context(tc.tile_pool(name="psp", bufs=2, space="PSUM"))
        oaccp = ctx.enter_context(tc.tile_pool(name="oaccp", bufs=2, space="PSUM"))

        # Pull the Exp activation table load off the critical path.
        dummy = singles.tile([1, 2], F32)
        nc.vector.memset(dummy, 0.0)
        nc.scalar.activation(dummy, dummy, AFT.Exp)

        sb_xT = singles.tile([128, 2, KT_D, 1024], F16)
        sb_xkT = singles.tile([128, nkc, KT_D, 512], F16)
        sb_wq = singles.tile([128, KT_D, 192], F16)
        sb_wk = singles.tile([128, KT_D, 192], F16)
        sb_wv = singles.tile([128, KT_D, 192], F16)
        sb_wo = singles.tile([128, 2, D], F16)
        sb_bq = singles.tile([128, 2], F32)
        sb_bk = singles.tile([128, 2], F32)
        sb_v = singles.tile([128, NH, nk_t, 65], F32R)
        sb_qT = singles.tile([128, 2, S], F16)
        sb_kT = singles.tile([128, 2, nkp], F16)
        sb_onT = singles.tile([128, 2, S], F16)

        # ---- DMA: scalar HW queue feeds the k path (its exp work starts
        # later); sync HW queue the q path + output; gpsimd the rest.
        nc.scalar.dma_start(out=sb_wk, in_=d_wk[:, :, :])
        for c in range(nkc):
            nc.scalar.dma_start(out=sb_xkT[:, c], in_=d_xkT[:, c])
        nc.sync.dma_start(out=sb_wq, in_=d_wq[:, :, :])
        nc.sync.dma_start(out=sb_xT[:, 0], in_=d_xT[:, 0])
        nc.sync.dma_start(out=sb_xT[:, 1], in_=d_xT[:, 1])
        nc.sync.dma_start(out=sb_wo, in_=d_wo[:, :, :])
        nc.gpsimd.dma_start(out=sb_bk, in_=d_bk[:, :])
        nc.gpsimd.dma_start(out=sb_bq, in_=d_bq[:, :])
        nc.gpsimd.dma_start(out=sb_wv, in_=d_wv[:, :, :])
        for h in range(NH):
            nc.gpsimd.dma_start(out=sb_v[:, h, :, 64:65], in_=d_vcol[:, :])

        # ---- k projection (512-col chunks) ----
        for c in range(nkc):
            for m, rows in enumerate((128, 64)):
                msl = ds(m * 128, rows)
                ps = psp.tile([128, 512], F32, tag="ps")
                for kt in range(KT_D):
                    nc.tensor.matmul(
                        ps[:rows, :],
                        sb_wk[:, kt, msl],
                        sb_xkT[:, c, kt, :],
                        start=(kt == 0),
                        stop=(kt == KT_D - 1),
                    )
                nc.vector.tensor_scalar_add(
                    out=sb_kT[:rows, m, ds(c * 512, 512)],
                    in0=ps[:rows, :],
                    scalar1=sb_bk[:rows, m : m + 1],
                )

        # ---- v projection (per 128-key tile, all 3 heads at once) ----
        for kt2 in range(nk_t):
            c, off = (kt2 * 128) // 512, (kt2 * 128) % 512
            ps = psp.tile([128, 192], F32, tag="ps")
            for kt in range(KT_D):
                nc.tensor.matmul(
                    ps,
                    sb_xkT[:, c, kt, ds(off, 128)],
                    sb_wv[:, kt, :],
                    start=(kt == 0),
                    stop=(kt == KT_D - 1),
                )
            nc.vector.tensor_copy(
                out=sb_v[:, :, kt2, 0:64],
                in_=ps[:, 0:192].rearrange("p (h d) -> p h d", h=NH),
            )

        # ---- q projection unit: one (chunk, slot) -> 512 cols x 2 ----
        def q_proj_unit(c, m):
            rows = 128 if m == 0 else 64
            msl = ds(m * 128, rows)
            ps = psp.tile([128, 1024], F32, tag="ps")
            for qc in range(2):
                for kt in range(KT_D):
                    nc.tensor.matmul(
                        ps[:rows, ts(qc, 512)],
                        sb_wq[:, kt, msl],
                        sb_xT[:, c, kt, ts(qc, 512)],
                        start=(kt == 0),
                        stop=(kt == KT_D - 1),
                    )
            nc.vector.tensor_scalar_add(
                out=sb_qT[:rows, m, ds(c * 1024, 1024)],
                in0=ps[:rows, :],
                scalar1=sb_bq[:rows, m : m + 1],
            )

        # ---- output projection: one query tile of 128 into its group ----
        def oproj_qt(qt, og, j):
            ps = psp.tile([128, 1024], F32, tag="ps")
            for eoff, ech in ((0, 512), (512, 256)):
                nc.tensor.matmul(
                    ps[:, ds(eoff, ech)],
                    sb_onT[:, 0, ts(qt, 128)],
                    sb_wo[:, 0, ds(eoff, ech)],
                    start=True,
                    stop=False,
                )
                nc.tensor.matmul(
                    ps[:, ds(eoff, ech)],
                    sb_onT[0:64, 1, ts(qt, 128)],
                    sb_wo[0:64, 1, ds(eoff, ech)],
                    start=False,
                    stop=True,
                )
            nc.vector.tensor_copy(out=og[:, j, :], in_=ps[:, 0:D])

        # ---- one attention head over one 1024-query half ----
        # fillers: {kt2: callable} of PE work to interleave after attnv so
        # the PE never idles (and never drops out of the fast pstate) while
        # ScalarE streams the exps.
        def attn_head(half, h, fillers=None):
            qrow = (h % 2) * 64
            qslot = h // 2
            oacc = oaccp.tile([65, 1024], F32, tag="oacc")
            for kt2 in range(nk_t):
                sT = psp.tile([128, 1024], F32, tag="ps")
                for qc in range(2):
                    nc.tensor.matmul(
                        sT[:, ts(qc, 512)],
                        sb_kT[ds(qrow, 64), qslot, ts(kt2, 128)],
                        sb_qT[ds(qrow, 64), qslot, ds(half * 1024 + qc * 512, 512)],
                        start=True,
                        stop=True,
                    )
                et = expp.tile([128, 1024], F32R, tag="exp")
                nc.scalar.activation(et, sT, AFT.Exp)
                for qc in range(2):
                    nc.tensor.matmul(
                        oacc[:, ts(qc, 512)],
                        sb_v[:, h, kt2, :],
                        et[:, ts(qc, 512)],
                        start=(kt2 == 0),
                        stop=(kt2 == nk_t - 1),
                    )
                if fillers and kt2 in fillers:
                    fillers[kt2]()
            # normalize in 512-col chunks: o.T[0:64]/denom(row 64) -> sb_onT
            for ch in range(2):
                csl = ds(ch * 512, 512)
                drow = rowp.tile([1, 512], F32, tag="drow")
                nc.vector.tensor_copy(out=drow, in_=oacc[64:65, csl])
                rrow = rowp.tile([1, 512], F32, tag="rrow")
                nc.vector.reciprocal_approx_fast(out=rrow, in_=drow)
                bcast = rowp.tile([64, 512], F32, tag="bcast")
                nc.gpsimd.partition_broadcast(bcast, rrow)
                nc.vector.tensor_mul(
                    out=sb_onT[ds(qrow, 64), qslot, ds(half * 1024 + ch * 512, 512)],
                    in0=oacc[0:64, csl],
                    in1=bcast,
                )

        # ---- schedule ----
        q_proj_unit(0, 0)
        q_proj_unit(0, 1)

        # head (0,0): interleave the second-half q projection as filler
        attn_head(
            0,
            0,
            fillers={
                2: lambda: q_proj_unit(1, 0),
                4: lambda: q_proj_unit(1, 1),
            },
        )
        attn_head(0, 1)
        attn_head(0, 2)

        # half 0's output projection rides inside half 1's attention
        og_state = {}

        def oproj_filler(qt, g, j, dma_after):
            def run():
                if j == 0:
                    og_state[g] = outp.tile([128, 4, D], F16, tag="og", name="og")
                oproj_qt(qt, og_state[g], j)
                if dma_after:
                    nc.sync.dma_start(
                        out=d_out[:, ds(g * 4 + 4 - dma_after, dma_after), :],
                        in_=og_state[g][:, ds(4 - dma_after, dma_after), :],
                    )
            return run

        f10 = {}
        for i, kt2 in enumerate((2, 3, 4, 5, 6, 7)):
            qt = i  # qt 0..5
            f10[kt2] = oproj_filler(qt, qt // 4, qt % 4, 4 if qt == 3 else 0)
        attn_head(1, 0, fillers=f10)
        f11 = {0: oproj_filler(6, 1, 2, 0), 1: oproj_filler(7, 1, 3, 4)}
        attn_head(1, 1, fillers=f11)
        attn_head(1, 2)

        # half 1's output projection: the tail; DMA in pairs so the last
        # transfer is small
        for g in (2, 3):
            og = outp.tile([128, 4, D], F16, tag="og")
            for j in range(4):
                oproj_qt(8 + (g - 2) * 4 + j, og, j)
                if j == 1 or j == 3:
                    nc.sync.dma_start(
                        out=d_out[:, ds(g * 4 + j - 1, 2), :],
                        in_=og[:, ds(j - 1, 2), :],
                    )

    nc.compile()
    return nc


def kernel(
    hidden_states,
    complexity_scores,
    attention_mask,
    Wq,
    bq,
    Wk,
    bk,
    Wv,
    bv,
    Wo,
    bo,
    emb_table,
    comp_scaling,
):
    global LAST_EXEC_TIME_NS, LAST_RESULTS
    hs = np.asarray(hidden_states, np.float32)
    cs = np.asarray(complexity_scores).astype(np.int64)
    am = np.asarray(attention_mask)
    Wq = np.asarray(Wq, np.float32)
    bq = np.asarray(bq, np.float32)
    Wk = np.asarray(Wk, np.float32)
    bk = np.asarray(bk, np.float32)
    Wv = np.asarray(Wv, np.float32)
    bv = np.asarray(bv, np.float32)
    Wo = np.asarray(Wo, np.float32)
    bo = np.asarray(bo, np.float32)
    emb_table = np.asarray(emb_table, np.float32)
    comp_scaling = np.asarray(comp_scaling, np.float32)

    # per-head score scale (identical across batch: mean over batch of embs)
    embs = emb_table[cs]  # (B, H)
    scal = comp_scaling * embs.mean(axis=0)  # (H,)
    c = (scal / math.sqrt(HD)).astype(np.float32)

    # gather unmasked keys per batch; pad to a common multiple of 128
    idx = [np.nonzero(am[b] != 0)[0] for b in range(B)]
    n_max = max(1, max(len(i) for i in idx))
    nk_t = max(2, (n_max + 127) // 128)
    n_k = nk_t * 128
    nkc = (n_k + 511) // 512
    nkp = nkc * 512

    xT = []
    xkT = []
    vcol = []
    for b in range(B):
        t = hs[b].T.astype(np.float16)  # (768, 2048)
        xT.append(
            np.ascontiguousarray(
                t.reshape(KT_D, 128, 2, 1024).transpose(1, 2, 0, 3)
            )
        )
        tk = np.zeros((D, nkp), np.float16)
        tk[:, : len(idx[b])] = hs[b][idx[b]].T
        xkT.append(
            np.ascontiguousarray(
                tk.reshape(KT_D, 128, nkc, 512).transpose(1, 2, 0, 3)
            )
        )
        v = np.zeros((nk_t * 128,), np.float32)
        v[: len(idx[b])] = 1.0
        vcol.append(np.ascontiguousarray(v.reshape(nk_t, 128).T))

    WqT = Wq.T  # (d_in, e_out)
    WkT = Wk.T
    WvT = Wv.T
    WoT = np.ascontiguousarray(Wo.T)  # rows = attended feature d

    def pack_w(w192):  # (768, 192) -> (128, KT_D, 192)
        return np.ascontiguousarray(
            w192.astype(np.float16).reshape(KT_D, 128, 192).transpose(1, 0, 2)
        )

    def pack_bias(vec):  # (192,) -> (128, 2)
        out = np.zeros((128, 2), np.float32)
        out[:, 0] = vec[:128]
        out[:64, 1] = vec[128:]
        return out

    in_maps = []
    for core in range(8):
        b = core // 4
        heads = [3 * (core % 4) + j for j in range(NH)]
        cols = np.concatenate([np.arange(h * HD, (h + 1) * HD) for h in heads])
        cscale = np.repeat(c[heads], HD)  # (192,)
        wq_c = pack_w(WqT[:, cols] * cscale[None, :])
        bq_c = bq[cols] * cscale
        wk_c = pack_w(WkT[:, cols])
        bk_c = bk[cols]
        wv_c = pack_w(WvT[:, cols])
        wo_c = np.zeros((128, 2, D), np.float16)
        wo_c[:, 0, :] = WoT[cols[:128], :]
        wo_c[:64, 1, :] = WoT[cols[128:], :]
        in_maps.append(
            {
                "xT": xT[b],
                "xkT": xkT[b],
                "wq": wq_c,
                "wk": wk_c,
                "wv": wv_c,
                "wo": np.ascontiguousarray(wo_c),
                "bq": pack_bias(bq_c),
                "bk": pack_bias(bk_c),
                "vcol": vcol[b],
            }
        )

    nc = build_nc(nk_t)
    trace = os.environ.get("KERNEL_TRACE", "0") == "1"
    res = run_bass_kernel_spmd(nc, in_maps, core_ids=list(range(8)), trace=trace)
    LAST_EXEC_TIME_NS = res.exec_time_ns
    LAST_RESULTS = res

    bo_eff = (bo + Wo @ bv).astype(np.float64)
    out = np.empty((B, S, D), np.float32)
    for b in range(B):
        acc = np.zeros((S, D), np.float64)
        for g in range(4):
            p = res.results[4 * b + g]["out"]  # (128, 16, D) fp16
            acc += p.astype(np.float64).transpose(1, 0, 2).reshape(S, D)
        out[b] = (acc + bo_eff[None, :]).astype(np.float32)
    return out


# revision 8
# speedup vs baseline: 1.2094x; 1.2094x over previous
"""ComplexityAwareAttention Trainium2 Bass kernel.

Sharding: 8 cores = 2 batches x 4 head-groups (3 heads each). Each core
computes q/k/v projections for its 3 heads, masked-key-gathered attention
(keys with attention_mask==0 are removed on host - softmax over the kept
keys is mathematically identical), and a partial output projection
(2048, 768). Host sums the 4 partials per batch and adds the fused output
bias (bo + Wo @ bv).

Numerics: fp16 for x / projection weights / q / k / onT / Wo / output
partials (validated 1.5e-3 rel err vs the f32 reference); f32r for the
exp'd scores and v (exp output can exceed fp16 range); f32 PSUM.

Scheduling: the attention inner loop is gated by ScalarE's exp
throughput; PE micro-stalls there drop it to the mid pstate (1.2 GHz)
and double the matmul cost. To keep the PE continuously busy it gets
filler matmul units interleaved one-per-iteration: q-projection of the
second query half inside the first attention head, and the first half's
output projection inside the second half's attention heads.
Normalization runs in 512-col chunks (copy PSUM denom row -> DVE
reciprocal_approx_fast -> Pool partition_broadcast -> DVE multiply) so
the final output projection starts per-chunk instead of per-half.
All HBM tensors are pre-packed on host into the exact SBUF layout
(partition-major) so every DMA moves large contiguous per-partition runs.
Only the Exp activation table is ever loaded (preloaded by a dummy exp
at t=0). No max-subtraction in softmax (|scores| << 80, exp cannot
overflow in f32).
"""

import math
import os
from contextlib import ExitStack

import numpy as np

import concourse.bass as bass
from concourse import bacc
import concourse.mybir as mybir
import concourse.tile as tile
from concourse.bass import ds, ts
from concourse.bass_utils import run_bass_kernel_spmd

F32 = mybir.dt.float32
F32R = mybir.dt.float32r
F16 = mybir.dt.float16
AFT = mybir.ActivationFunctionType

B = 2
S = 2048
D = 768
H = 12
HD = 64
NH = 3  # heads per core
KT_D = D // 128  # 6 contraction tiles over d_model

LAST_EXEC_TIME_NS = None
LAST_RESULTS = None


def build_nc(nk_t):
    n_k = nk_t * 128
    nkc = (n_k + 511) // 512  # xkT 512-col chunks
    nkp = nkc * 512  # padded key columns
    nc = bacc.Bacc(None, target_bir_lowering=False)

    d_xT = nc.dram_tensor("xT", (128, 2, KT_D, 1024), F16, kind="ExternalInput")
    d_xkT = nc.dram_tensor("xkT", (128, nkc, KT_D, 512), F16, kind="ExternalInput")
    d_wq = nc.dram_tensor("wq", (128, KT_D, 192), F16, kind="ExternalInput")
    d_wk = nc.dram_tensor("wk", (128, KT_D, 192), F16, kind="ExternalInput")
    d_wv = nc.dram_tensor("wv", (128, KT_D, 192), F16, kind="ExternalInput")
    d_wo = nc.dram_tensor("wo", (128, 2, D), F16, kind="ExternalInput")
    d_bq = nc.dram_tensor("bq", (128, 2), F32, kind="ExternalInput")
    d_bk = nc.dram_tensor("bk", (128, 2), F32, kind="ExternalInput")
    d_vcol = nc.dram_tensor("vcol", (128, nk_t), F32R, kind="ExternalInput")
    d_out = nc.dram_tensor("out", (128, 16, D), F16, kind="ExternalOutput")

    with ExitStack() as ctx:
        tc = ctx.enter_context(tile.TileContext(nc))
        singles = ctx.enter_context(tc.tile_pool(name="singles", bufs=1))
        expp = ctx.enter_context(tc.tile_pool(name="expp", bufs=4))
        outp = ctx.enter_context(tc.tile_pool(name="outp", bufs=2))
        rowp = ctx.enter_context(tc.tile_pool(name="rowp", bufs=2))
        psp = ctx.enter_context(tc.tile_pool(name="psp", bufs=2, space="PSUM"))
        oaccp = ctx.enter_context(tc.tile_pool(name="oaccp", bufs=2, space="PSUM"))

        # Pull the Exp activation table load off the critical path.
        dummy = singles.tile([1, 2], F32)
        nc.vector.memset(dummy, 0.0)
        nc.scalar.activation(dummy, dummy, AFT.Exp)

        sb_xT = singles.tile([128, 2, KT_D, 1024], F16)
        sb_xkT = singles.tile([128, nkc, KT_D, 512], F16)
        sb_wq = singles.tile([128, KT_D, 192], F16)
        sb_wk = singles.tile([128, KT_D, 192], F16)
        sb_wv = singles.tile([128, KT_D, 192], F16)
        sb_wo = singles.tile([128, 2, D], F16)
        sb_bq = singles.tile([128, 2], F32)
        sb_bk = singles.tile([128, 2], F32)
        sb_v = singles.tile([128, NH, nk_t, 65], F32R)
        sb_qT = singles.tile([128, 2, S], F16)
        sb_kT = singles.tile([128, 2, nkp], F16)
        sb_onT = singles.tile([128, 2, S], F16)

        # ---- DMA: scalar HW queue feeds the k path (its exp work starts
        # later); sync HW queue the q path + output; gpsimd the rest.
        nc.scalar.dma_start(out=sb_wk, in_=d_wk[:, :, :])
        for c in range(nkc):
            nc.scalar.dma_start(out=sb_xkT[:, c], in_=d_xkT[:, c])
        nc.gpsimd.dma_start(out=sb_wv, in_=d_wv[:, :, :])
        nc.gpsimd.dma_start(out=sb_wq, in_=d_wq[:, :, :])
        nc.sync.dma_start(out=sb_bq, in_=d_bq[:, :])
        nc.sync.dma_start(out=sb_bk, in_=d_bk[:, :])
        for h in range(NH):
            nc.sync.dma_start(out=sb_v[:, h, :, 64:65], in_=d_vcol[:, :])
        nc.sync.dma_start(out=sb_xT[:, 0], in_=d_xT[:, 0])
        nc.sync.dma_start(out=sb_xT[:, 1], in_=d_xT[:, 1])
        nc.sync.dma_start(out=sb_wo, in_=d_wo[:, :, :])

        # ---- k projection (512-col chunks) ----
        for c in range(nkc):
            for m, rows in enumerate((128, 64)):
                msl = ds(m * 128, rows)
                ps = psp.tile([128, 512], F32, tag="ps")
                for kt in range(KT_D):
                    nc.tensor.matmul(
                        ps[:rows, :],
                        sb_wk[:, kt, msl],
                        sb_xkT[:, c, kt, :],
                        start=(kt == 0),
                        stop=(kt == KT_D - 1),
                    )
                nc.vector.tensor_scalar_add(
                    out=sb_kT[:rows, m, ds(c * 512, 512)],
                    in0=ps[:rows, :],
                    scalar1=sb_bk[:rows, m : m + 1],
                )

        # ---- v projection (per 128-key tile, all 3 heads at once) ----
        for kt2 in range(nk_t):
            c, off = (kt2 * 128) // 512, (kt2 * 128) % 512
            ps = psp.tile([128, 192], F32, tag="ps")
            for kt in range(KT_D):
                nc.tensor.matmul(
                    ps,
                    sb_xkT[:, c, kt, ds(off, 128)],
                    sb_wv[:, kt, :],
                    start=(kt == 0),
                    stop=(kt == KT_D - 1),
                )
            nc.vector.tensor_copy(
                out=sb_v[:, :, kt2, 0:64],
                in_=ps[:, 0:192].rearrange("p (h d) -> p h d", h=NH),
            )

        # ---- q projection unit: one (chunk, slot) -> 512 cols x 2 ----
        def q_proj_unit(c, m):
            rows = 128 if m == 0 else 64
            msl = ds(m * 128, rows)
            ps = psp.tile([128, 1024], F32, tag="ps")
            for qc in range(2):
                for kt in range(KT_D):
                    nc.tensor.matmul(
                        ps[:rows, ts(qc, 512)],
                        sb_wq[:, kt, msl],
                        sb_xT[:, c, kt, ts(qc, 512)],
                        start=(kt == 0),
                        stop=(kt == KT_D - 1),
                    )
            nc.vector.tensor_scalar_add(
                out=sb_qT[:rows, m, ds(c * 1024, 1024)],
                in0=ps[:rows, :],
                scalar1=sb_bq[:rows, m : m + 1],
            )

        # ---- output projection: one query tile of 128 into its group ----
        def oproj_qt(qt, og, j):
            ps = psp.tile([128, 1024], F32, tag="ps")
            for eoff, ech in ((0, 512), (512, 256)):
                nc.tensor.matmul(
                    ps[:, ds(eoff, ech)],
                    sb_onT[:, 0, ts(qt, 128)],
                    sb_wo[:, 0, ds(eoff, ech)],
                    start=True,
                    stop=False,
                )
                nc.tensor.matmul(
                    ps[:, ds(eoff, ech)],
                    sb_onT[0:64, 1, ts(qt, 128)],
                    sb_wo[0:64, 1, ds(eoff, ech)],
                    start=False,
                    stop=True,
                )
            nc.vector.tensor_copy(out=og[:, j, :], in_=ps[:, 0:D])

        # ---- one attention head over one 1024-query half ----
        # fillers: {kt2: callable} of PE work to interleave after attnv so
        # the PE never idles (and never drops out of the fast pstate) while
        # ScalarE streams the exps.
        def attn_head(half, h, fillers=None):
            qrow = (h % 2) * 64
            qslot = h // 2
            oacc = oaccp.tile([65, 1024], F32, tag="oacc")
            for kt2 in range(nk_t):
                sT = psp.tile([128, 1024], F32, tag="ps")
                for qc in range(2):
                    nc.tensor.matmul(
                        sT[:, ts(qc, 512)],
                        sb_kT[ds(qrow, 64), qslot, ts(kt2, 128)],
                        sb_qT[ds(qrow, 64), qslot, ds(half * 1024 + qc * 512, 512)],
                        start=True,
                        stop=True,
                    )
                et = expp.tile([128, 1024], F32R, tag="exp")
                nc.scalar.activation(et, sT, AFT.Exp)
                for qc in range(2):
                    nc.tensor.matmul(
                        oacc[:, ts(qc, 512)],
                        sb_v[:, h, kt2, :],
                        et[:, ts(qc, 512)],
                        start=(kt2 == 0),
                        stop=(kt2 == nk_t - 1),
                    )
                if fillers and kt2 in fillers:
                    fillers[kt2]()
            # normalize in 512-col chunks: o.T[0:64]/denom(row 64) -> sb_onT
            for ch in range(2):
                csl = ds(ch * 512, 512)
                drow = rowp.tile([1, 512], F32, tag="drow")
                nc.vector.tensor_copy(out=drow, in_=oacc[64:65, csl])
                rrow = rowp.tile([1, 512], F32, tag="rrow")
                nc.vector.reciprocal_approx_fast(out=rrow, in_=drow)
                bcast = rowp.tile([64, 512], F32, tag="bcast")
                nc.gpsimd.partition_broadcast(bcast, rrow)
                nc.vector.tensor_mul(
                    out=sb_onT[ds(qrow, 64), qslot, ds(half * 1024 + ch * 512, 512)],
                    in0=oacc[0:64, csl],
                    in1=bcast,
                )

        # ---- schedule: keep a standing matmul backlog on the PE through
        # the exp-gated attention phase (micro-stalls invite the DVFS
        # governor to clamp the PE to half speed), so the deferrable PE
        # blocks (q half 1, half 0's output projection) are placed as late
        # as their consumers allow.
        q_proj_unit(0, 0)
        q_proj_unit(0, 1)
        attn_head(0, 0)
        q_proj_unit(1, 0)
        q_proj_unit(1, 1)
        attn_head(0, 1)
        attn_head(0, 2)
        attn_head(1, 0)
        attn_head(1, 1)

        def oproj_block(half, last):
            for g in (half * 2, half * 2 + 1):
                og = outp.tile([128, 4, D], F16, tag="og", name="og")
                for j in range(4):
                    oproj_qt(g * 4 + j, og, j)
                    if last and g == half * 2 + 1 and j in (1, 3):
                        # tail: DMA in pairs so the final transfer is small
                        nc.sync.dma_start(
                            out=d_out[:, ds(g * 4 + j - 1, 2), :],
                            in_=og[:, ds(j - 1, 2), :],
                        )
                if not (last and g == half * 2 + 1):
                    nc.sync.dma_start(out=d_out[:, ds(g * 4, 4), :], in_=og)

        oproj_block(0, last=False)
        attn_head(1, 2)
        oproj_block(1, last=True)

    nc.compile()
    return nc


def kernel(
    hidden_states,
    complexity_scores,
    attention_mask,
    Wq,
    bq,
    Wk,
    bk,
    Wv,
    bv,
    Wo,
    bo,
    emb_table,
    comp_scaling,
):
    global LAST_EXEC_TIME_NS, LAST_RESULTS
    hs = np.asarray(hidden_states, np.float32)
    cs = np.asarray(complexity_scores).astype(np.int64)
    am = np.asarray(attention_mask)
    Wq = np.asarray(Wq, np.float32)
    bq = np.asarray(bq, np.float32)
    Wk = np.asarray(Wk, np.float32)
    bk = np.asarray(bk, np.float32)
    Wv = np.asarray(Wv, np.float32)
    bv = np.asarray(bv, np.float32)
    Wo = np.asarray(Wo, np.float32)
    bo = np.asarray(bo, np.float32)
    emb_table = np.asarray(emb_table, np.float32)
    comp_scaling = np.asarray(comp_scaling, np.float32)

    # per-head score scale (identical across batch: mean over batch of embs)
    embs = emb_table[cs]  # (B, H)
    scal = comp_scaling * embs.mean(axis=0)  # (H,)
    c = (scal / math.sqrt(HD)).astype(np.float32)

    # gather unmasked keys per batch; pad to a common multiple of 128
    idx = [np.nonzero(am[b] != 0)[0] for b in range(B)]
    n_max = max(1, max(len(i) for i in idx))
    nk_t = max(2, (n_max + 127) // 128)
    n_k = nk_t * 128
    nkc = (n_k + 511) // 512
    nkp = nkc * 512

    xT = []
    xkT = []
    vcol = []
    for b in range(B):
        t = hs[b].T.astype(np.float16)  # (768, 2048)
        xT.append(
            np.ascontiguousarray(
                t.reshape(KT_D, 128, 2, 1024).transpose(1, 2, 0, 3)
            )
        )
        tk = np.zeros((D, nkp), np.float16)
        tk[:, : len(idx[b])] = hs[b][idx[b]].T
        xkT.append(
            np.ascontiguousarray(
                tk.reshape(KT_D, 128, nkc, 512).transpose(1, 2, 0, 3)
            )
        )
        v = np.zeros((nk_t * 128,), np.float32)
        v[: len(idx[b])] = 1.0
        vcol.append(np.ascontiguousarray(v.reshape(nk_t, 128).T))

    WqT = Wq.T  # (d_in, e_out)
    WkT = Wk.T
    WvT = Wv.T
    WoT = np.ascontiguousarray(Wo.T)  # rows = attended feature d

    def pack_w(w192):  # (768, 192) -> (128, KT_D, 192)
        return np.ascontiguousarray(
            w192.astype(np.float16).reshape(KT_D, 128, 192).transpose(1, 0, 2)
        )

    def pack_bias(vec):  # (192,) -> (128, 2)
        out = np.zeros((128, 2), np.float32)
        out[:, 0] = vec[:128]
        out[:64, 1] = vec[128:]
        return out

    in_maps = []
    for core in range(8):
        b = core // 4
        heads = [3 * (core % 4) + j for j in range(NH)]
        cols = np.concatenate([np.arange(h * HD, (h + 1) * HD) for h in heads])
        cscale = np.repeat(c[heads], HD)  # (192,)
        wq_c = pack_w(WqT[:, cols] * cscale[None, :])
        bq_c = bq[cols] * cscale
        wk_c = pack_w(WkT[:, cols])
        bk_c = bk[cols]
        wv_c = pack_w(WvT[:, cols])
        wo_c = np.zeros((128, 2, D), np.float16)
        wo_c[:, 0, :] = WoT[cols[:128], :]
        wo_c[:64, 1, :] = WoT[cols[128:], :]
        in_maps.append(
            {
                "xT": xT[b],
                "xkT": xkT[b],
                "wq": wq_c,
                "wk": wk_c,
                "wv": wv_c,
                "wo": np.ascontiguousarray(wo_c),
                "bq": pack_bias(bq_c),
                "bk": pack_bias(bk_c),
                "vcol": vcol[b],
            }
        )

    nc = build_nc(nk_t)
    trace = os.environ.get("KERNEL_TRACE", "0") == "1"
    res = run_bass_kernel_spmd(nc, in_maps, core_ids=list(range(8)), trace=trace)
    LAST_EXEC_TIME_NS = res.exec_time_ns
    LAST_RESULTS = res

    bo_eff = (bo + Wo @ bv).astype(np.float64)
    out = np.empty((B, S, D), np.float32)
    for b in range(B):
        acc = np.zeros((S, D), np.float64)
        for g in range(4):
            p = res.results[4 * b + g]["out"]  # (128, 16, D) fp16
            acc += p.astype(np.float64).transpose(1, 0, 2).reshape(S, D)
        out[b] = (acc + bo_eff[None, :]).astype(np.float32)
    return out
